# revision 54
# baseline (speedup 1.0000x reference)
"""
BiRNN Trainium2 kernel (8 NeuronCores, SPMD).

Problem: x:[64,512,64], bidirectional sigmoid RNN with H=1024, out O=512.
    xp = x @ Wx + bx                      (per time step)
    f_t = sigmoid(xp_t + f_{t-1} @ Ws + bs)   forward scan
    b_t = sigmoid(xp_t + b_{t+1} @ Ws + bs)   backward scan
    out = (f + b) @ Wout + bout

Strategy (speculative time-sharding, transposed-space fp8 recurrence):
  The scan map h -> sigmoid(xp + h@Ws + bs) is strongly contractive
  (~0.35x/step), so each of the 8 cores takes one 64-step time chunk and
  recomputes it from a warm-started state (W warmup steps), running BOTH
  directions fused (64 fwd + 64 bwd chains = 128 moving columns).
  Boundary chains (core 0 fwd, core 7 bwd) get "walled" warmups
  (pre forced to -240/16 -> state ~= saturated) and the true h0 @ Ws
  contribution is folded in via one extra matmul sweep at s == W.

  The recurrence runs in TRANSPOSED space: state [h, chain] is both the
  matmul moving operand and the activation output, so no PE transposes are
  needed anywhere. All recurrence matmuls are fp8e4 with
  MatmulPerfMode.DoubleRow (contraction-pair packing, 0.5 cycles/row).

  Precision: the state is stored as u = 2h - 1 = tanh(P/32) in fp8e4
  (P = 16*pre accumulates in PSUM f32). e4m3's error is proportional to
  magnitude, and |u| is smallest exactly where the sigmoid is sensitive,
  halving state noise vs storing h. The identity h@Ws = u@(Ws*8)/16 +
  (Ws.sum(0)*8)/16 folds the offset into the bias, carried by TWO fp8
  ones-rows (hi + lo split) of the input projection. Weights are
  pre-scaled (Wx*16, Ws*8) into e4m3's normal range.

  Per-step schedule (the steady-state cycle is tanh-latency-bound):
  PSUM bank A holds pre-blocks j0-3, bank B j4-7 (as two sequential
  accumulation groups j4-5 / j6-7 riding one lazy-zero region). The big
  tanh covers h-chunks 0-5, the small one 6-7, so after the small tanh of
  step s only the 8 k-pair-3 matmuls gate the bank stops of s+1, and the
  p0-p2 sweeps hide under the big tanh's own latency window.

  Output: state blocks (two steps per tile) stay resident in SBUF; for a
  timestep pair, s_hi = fp8(uf + ub) via two direct DVE adds, then ONE
  fp8-DoubleRow 2-term projection per pair (s_hi@wo_hi + s_hi@wo_lo with
  wo_hi+wo_lo = 8*Wout in fp8, M = 2 timesteps x 64 batch), and
  st = po/16 + (Wout.sum(0) + bout) restores h-units. Projections are
  emitted one step late to fill the PE stream (which idles on the tanh
  chain) instead of stalling the recurrence.
"""

import sys
from contextlib import ExitStack

import numpy as np
import ml_dtypes

if "/opt/trn_rl_repo" not in sys.path:
    sys.path.insert(0, "/opt/trn_rl_repo")

import concourse.bass as bass  # noqa: E402
import concourse.mybir as mybir  # noqa: E402
import concourse.tile as tile  # noqa: E402
from concourse import bacc  # noqa: E402
from concourse.bass_utils import run_bass_kernel_spmd  # noqa: E402
from concourse.masks import make_identity  # noqa: E402

F32 = mybir.dt.float32
BF16 = mybir.dt.bfloat16
F8 = mybir.dt.float8e4
DR = mybir.MatmulPerfMode.DoubleRow
NP_F8 = ml_dtypes.float8_e4m3
NP_BF16 = ml_dtypes.bfloat16

B, T, I, H, O = 64, 512, 64, 1024, 512
NCORES = 8
C = T // NCORES          # 64: time-chunk per core
W = 2                    # speculative warmup steps (even: state-block pair)
S = W + C                # steps per core
KC = H // 128            # 8 h-blocks
NPAIR = C // 2           # 32 output pairs per core
KX = 34                  # xt partition rows (2x34 = 68: 64 x, wall, 2 bias)
WALL = -240.0            # P-scale wall; tanh(-240/32) = -1 exactly in fp8

_BUILD_CACHE = None


def _pairs_done_at():
    """For each step s, the list of output pairs whose second direction
    lands at s (both t=2p and t=2p+1 complete for fwd and bwd)."""
    done = {s: [] for s in range(S)}
    for p in range(NPAIR):
        s = W + max(2 * p + 1, C - 1 - 2 * p)
        done[s].append(p)
    return done


def _build_program():
    """Build + compile the (SPMD-uniform) Bass program once."""
    global _BUILD_CACHE
    if _BUILD_CACHE is not None:
        return _BUILD_CACHE

    nc = bacc.Bacc("TRN2", target_bir_lowering=False, debug=False,
                   num_devices=NCORES)

    xt_d = nc.dram_tensor("xt", [S, KX, 2, 128], F8,
                          kind="ExternalInput").ap()
    wxa_d = nc.dram_tensor("wxa", [KX, 2, KC, 128], F8,
                           kind="ExternalInput").ap()
    ws_d = nc.dram_tensor("ws", [128, 4, 2, KC, 128], F8,
                          kind="ExternalInput").ap()
    fold_d = nc.dram_tensor("fold", [128, KC, 128], BF16,
                            kind="ExternalInput").ap()
    wo_hi_d = nc.dram_tensor("wo_hi", [128, 4, 2, O], F8,
                             kind="ExternalInput").ap()
    wo_lo_d = nc.dram_tensor("wo_lo", [128, 4, 2, O], F8,
                             kind="ExternalInput").ap()
    bb_d = nc.dram_tensor("bb", [128, O], F32, kind="ExternalInput").ap()
    init_d = nc.dram_tensor("init", [128, KC, 128], F8,
                            kind="ExternalInput").ap()
    out_d = nc.dram_tensor("out", [NPAIR, 128, O], F32,
                           kind="ExternalOutput").ap()

    with tile.TileContext(nc) as tc, ExitStack() as ctx:
        const = ctx.enter_context(tc.tile_pool(name="const", bufs=1))
        # Load order matters: step 0 needs wxa/ws/init/xt[0] only; wo, fold
        # and bb trail behind the first steps' compute.
        wxa_s = const.tile([KX, 2, KC, 128], F8)
        nc.sync.dma_start(wxa_s[:], wxa_d[:])
        ws_s = const.tile([128, 4, 2, KC, 128], F8)
        init_s = const.tile([128, KC, 128], F8)

        xt_pool = ctx.enter_context(tc.tile_pool(name="xt", bufs=10))
        pre_pool = ctx.enter_context(
            tc.tile_pool(name="pre", bufs=2, space="PSUM"))
        h_pool = ctx.enter_context(tc.tile_pool(name="h", bufs=S + 1))
        s8_pool = ctx.enter_context(tc.tile_pool(name="s8", bufs=8))
        po_pool = ctx.enter_context(
            tc.tile_pool(name="po", bufs=2, space="PSUM"))
        st_pool = ctx.enter_context(tc.tile_pool(name="st", bufs=4))

        done_at = _pairs_done_at()
        blocks = {}
        xt_tiles = {}
        LOOK = 8

        def fetch_xt(step):
            t_ = xt_pool.tile([KX, 2, 128], F8, tag="xt")
            nc.sync.dma_start(t_[:], xt_d[step])
            xt_tiles[step] = t_

        fetch_xt(0)
        nc.sync.dma_start(ws_s[:, 0], ws_d[:, 0])
        nc.sync.dma_start(init_s[:], init_d[:])
        # Remaining weights stream in parallel on the GPSIMD-issued queue
        # while SP keeps feeding per-step xt tiles (ws3 back on SP so the
        # two queues land the four ws quarters at similar times).
        for p in range(1, 4):
            nc.gpsimd.dma_start(ws_s[:, p], ws_d[:, p])
        fold_s = const.tile([128, KC, 128], BF16)
        nc.gpsimd.dma_start(fold_s[:], fold_d[:])
        wo_hi_s = const.tile([128, 4, 2, O], F8)
        nc.gpsimd.dma_start(wo_hi_s[:], wo_hi_d[:])
        wo_lo_s = const.tile([128, 4, 2, O], F8)
        nc.gpsimd.dma_start(wo_lo_s[:], wo_lo_d[:])
        bb_s = const.tile([128, O], F32)
        nc.gpsimd.dma_start(bb_s[:], bb_d[:])
        for s in range(1, min(LOOK, S)):
            fetch_xt(s)
        ident = const.tile([128, 128], F32)
        make_identity(nc, ident[:])
        ident_bf = const.tile([128, 128], BF16)
        nc.scalar.copy(ident_bf[:], ident[:])

        def _fused_proj(p, s_hi):
            # fp8 DoubleRow 2-term projection: 8*sp@Wout ~= s_hi@wo_hi
            # + s_hi@wo_lo with s_hi = fp8(uf + ub) straight from the DVE
            # adds; st = po/16 + bb restores h-units.
            def emit():
                po_t = po_pool.tile([128, O], F32, tag="po")
                for ti, w_t in enumerate((wo_hi_s, wo_lo_s)):
                    for p4 in range(4):
                        nc.tensor.matmul(
                            po_t[:], s_hi[:, 2 * p4:2 * p4 + 2, :],
                            w_t[:, p4, :, :],
                            start=(ti == 0 and p4 == 0),
                            stop=(ti == 1 and p4 == 3), perf_mode=DR)
                st_t = st_pool.tile([128, O], F32, tag="st")
                nc.vector.scalar_tensor_tensor(
                    st_t[:], po_t[:], 1.0 / 16.0, bb_s[:],
                    mybir.AluOpType.mult, mybir.AluOpType.add)
                nc.sync.dma_start(out_d[p], st_t[:])
            return emit

        pending = []
        pending_next = []
        for s in range(S):
            if s + LOOK < S:
                fetch_xt(s + LOOK)
            xt_t = xt_tiles.pop(s)
            state_new = h_pool.tile([128, KC, 128], F8,
                                    name=f"st{s}", tag="h")
            blocks[s] = state_new
            pre = pre_pool.tile([128, KC, 128], F32, tag="pre")

            if s == 0:
                def prev_ap(p):
                    return init_s[:, 2 * p:2 * p + 2, :]
            else:
                pst = blocks[s - 1]

                def prev_ap(p):
                    return pst[:, 2 * p:2 * p + 2, :]

            # PSUM groups: G_lo = j0-3 (bank A), G_h1 = j4-5 and G_h2 = j6-7
            # (bank B; sequential groups in one zero region — G_h2 rides
            # G_h1's start=True lazy-zero with the group check skipped).
            # Activations split [6 + 2]: the big tanh covers chunks 0-5
            # (k-pairs 0-2), the small one chunks 6-7 (k-pair 3), so the
            # cross-step chain after the last tanh is only the 8 pair-3
            # matmuls, and p0-p2 hide under the big tanh's latency.
            def mm(j, lhsT, rhs, start=False, stop=False, dr=True):
                nc.tensor.matmul(pre[:, j, :], lhsT, rhs, start=start,
                                 stop=stop, skip_group_check=(j >= 6),
                                 perf_mode=DR if dr else None)

            for j in range(KC):
                mm(j, wxa_s[:, :, j, :], xt_t[:], start=(j in (0, 4)))
            for p in range(3):
                for j in range(KC):
                    mm(j, ws_s[:, p, :, j, :], prev_ap(p))
            if s == W:
                # Fold the true h0 @ Ws contribution into the first real step.
                for j in range(KC):
                    mm(j, ident_bf[:], fold_s[:, j, :], dr=False)
            for j in range(KC):
                mm(j, ws_s[:, 3, :, j, :], prev_ap(3),
                   stop=(j in (3, 5, 7)))

            # Projections queued from earlier steps: in the PE stream they
            # land after this step's scan matmuls, filling the tanh-wait
            # bubble instead of stalling the recurrence.
            for fn in pending:
                fn()
            pending = pending_next
            pending_next = []

            for lo, hi in ((0, 6), (6, 8)):
                nc.scalar.activation(state_new[:, lo:hi, :],
                                     pre[:, lo:hi, :],
                                     mybir.ActivationFunctionType.Tanh,
                                     scale=1.0 / 32.0)

            # Output pairs completed at this step: sp = uf + ub via two
            # direct DVE adds from the resident state blocks (slot-matched:
            # the b block holds b_{2p+1}, b_{2p} at slots 0, 1).
            for qi, p in enumerate(done_at[s]):
                t0 = 2 * p
                s_hi = s8_pool.tile([128, KC, 128], F8, tag="s8h")
                nc.vector.tensor_add(s_hi[:, :, 0:64],
                                     blocks[W + t0][:, :, 0:64],
                                     blocks[W + C - 1 - t0][:, :, 64:128])
                nc.vector.tensor_add(s_hi[:, :, 64:128],
                                     blocks[W + t0 + 1][:, :, 0:64],
                                     blocks[W + C - 2 - t0][:, :, 64:128])
                (pending if qi == 0 else pending_next).append(
                    _fused_proj(p, s_hi))

        for fn in pending + pending_next:
            fn()

    nc.compile()
    _BUILD_CACHE = nc
    return nc


def _prepare_inputs(x, h0_f, h0_b, Wx, bx, Ws, bs, Wout, bout):
    """Host-side data marshaling: per-core input dicts."""
    x = np.ascontiguousarray(np.asarray(x, np.float32))
    h0_f = np.asarray(h0_f, np.float32)
    h0_b = np.asarray(h0_b, np.float32)
    Wx = np.asarray(Wx, np.float32)
    bx = np.asarray(bx, np.float32)
    Ws = np.asarray(Ws, np.float32)
    bs = np.asarray(bs, np.float32)
    Wout = np.asarray(Wout, np.float32)
    bout = np.asarray(bout, np.float32)

    # wxa rows r = sub*KX + part: 0-63 = Wx*16; 64 = wall; 65/66 = bias
    # hi/lo (P-scale bias = 16*(bx+bs) + 8*Ws.sum(0) from the u = 2h-1
    # substitution), fp8 hi+lo so the bias lands to ~0.1%.
    bias = 16.0 * (bx + bs) + 8.0 * Ws.sum(0)
    b_hi = bias.astype(NP_F8).astype(np.float32)
    b_lo = bias - b_hi
    wxa = np.zeros((2 * KX, KC, 128), np.float32)
    wxa[0:64] = (Wx * 16.0).reshape(64, KC, 128)
    wxa[64] = WALL
    wxa[65] = b_hi.reshape(KC, 128)
    wxa[66] = b_lo.reshape(KC, 128)
    wxa = np.ascontiguousarray(
        wxa.reshape(2, KX, KC, 128).transpose(1, 0, 2, 3))

    # ws[part, p, sub, j, m] = (Ws*8)[(2p+sub)*128 + part, j*128 + m]
    ws_l = np.ascontiguousarray(
        (Ws * 8.0).reshape(4, 2, 128, KC, 128).transpose(2, 0, 1, 3, 4))
    # wo8[part, p, sub, o] = (Wout*8)[(2p+sub)*128 + part, o]; fp8 hi + lo
    wo8 = np.ascontiguousarray(
        (Wout * 8.0).reshape(4, 2, 128, O).transpose(2, 0, 1, 3))
    wo_hi = wo8.astype(NP_F8)
    wo_lo = (wo8 - wo_hi.astype(np.float32)).astype(NP_F8)
    # st = po/16 + bb where bb = Wout.sum(0) + bout (u = 2h-1 offset)
    bb = np.ascontiguousarray(np.broadcast_to(
        Wout.sum(0) + bout, (128, O)).astype(np.float32))
    init = np.zeros((128, KC, 128), np.float32)  # u(h=0.5) = 0

    s_idx = np.arange(S)
    in_maps = []
    for c in range(NCORES):
        tf = 64 * c - W + s_idx            # fwd absolute times
        tb = 64 * c + (C - 1) + W - s_idx  # bwd absolute times
        ok_f = (tf >= 0) & (tf < T)
        ok_b = (tb >= 0) & (tb < T)
        xt = np.zeros((S, 2 * KX, 128), np.float32)
        # x[batch, t, i] -> rows i, cols = chains
        xf = x[:, np.clip(tf, 0, T - 1), :].transpose(1, 2, 0)  # [S, I, B]
        xb = x[:, np.clip(tb, 0, T - 1), :].transpose(1, 2, 0)
        xt[:, 0:64, 0:64] = xf * ok_f[:, None, None]
        xt[:, 0:64, 64:128] = xb * ok_b[:, None, None]
        xt[:, 65, :] = 1.0  # bias hi row
        xt[:, 66, :] = 1.0  # bias lo row
        # wall flags: only boundary chains' warmup steps
        if c == 0:
            xt[0:W, 64, 0:64] = 1.0
        if c == NCORES - 1:
            xt[0:W, 64, 64:128] = 1.0
        xt = np.ascontiguousarray(
            xt.reshape(S, 2, KX, 128).transpose(0, 2, 1, 3))

        # fold[h_part, j, chain] = (h0 @ Ws * 16)[chain, j*128 + h_part]
        fold = np.zeros((128, KC, 128), np.float32)
        if c == 0:
            fold[:, :, 0:64] = (
                (h0_f @ Ws) * 16).T.reshape(KC, 128, 64).transpose(1, 0, 2)
        if c == NCORES - 1:
            fold[:, :, 64:128] = (
                (h0_b @ Ws) * 16).T.reshape(KC, 128, 64).transpose(1, 0, 2)

        in_maps.append({
            "xt": xt.astype(NP_F8),
            "wxa": wxa.astype(NP_F8),
            "ws": ws_l.astype(NP_F8),
            "fold": fold.astype(NP_BF16),
            "wo_hi": wo_hi,
            "wo_lo": wo_lo,
            "bb": bb,
            "init": init.astype(NP_F8),
        })
    return in_maps


def _gather(results):
    full = np.empty((B, T, O), np.float32)
    for c in range(NCORES):
        o = results[c]["out"].reshape(NPAIR, 2, 64, O)
        # [pair, parity, batch, O] -> [batch, t', O]
        block = o.transpose(2, 0, 1, 3).reshape(64, C, O)
        full[:, 64 * c:64 * (c + 1), :] = block
    return full


def kernel(x, h0_f, h0_b, Wx, bx, Ws, bs, Wout, bout):
    nc = _build_program()
    in_maps = _prepare_inputs(x, h0_f, h0_b, Wx, bx, Ws, bs, Wout, bout)
    res = run_bass_kernel_spmd(nc, in_maps, core_ids=list(range(NCORES)))
    return _gather(res.results)
